# revision 77
# baseline (speedup 1.0000x reference)
"""Trainium2 Bass kernel for nn_MultiHeadCrossAttention_82033875354222.

Math (per batch b, with n = H*W = 4096, CN = 512, C = 64):
    Q = Wq q + bq ; K = Wk kv + bk ; V = Wv kv + bv          (1x1 convs)
    scores = Q K^T / 64 ; attn = softmax(scores, axis=-1)    ([512, 512])
    out = attn V                                             ([512, 4096])
    x2 = permute(0,2,1).reshape -> [512, H, W]               (pure relabel)
    y = w2 @ leaky(w1 @ leaky(BN(x2)) + b1) + b2

Key algebraic restructuring (v2):
  * rank-65 attention: scores^T = Wka (kva qa^T) Wqa^T / 64 is computed
    DIRECTLY in transposed [k, q] layout (no PE transposes needed), via
    MT = kva qa^T, X = MT^T Wka^T, scoresT chunks = X_chunk^T Wqa^T.
  * softmax normalization is deferred past the U matmul: with E = exp(scoresT)
    and wva augmented by a ones column, u_ps = Wva_aug^T E has row 65 equal to
    the softmax column sums s[q]; one reciprocal + partition-broadcast +
    multiply normalizes U^T and leaves row 65 == 1.
  * BN is folded into the x2 matmul: the BN scale is pre-multiplied into kp's
    columns on the host and the BN shift rides a 66th kp row that pairs with
    the (all-ones) row 65 of the normalized U^T. The x2 eviction is then a
    pure leaky-relu.
  * y2 = w2 @ y1 keeps w2T stationary (64 cols) and streams y1 (N=512),
    producing the output directly in [C, spatial] layout (the host-side
    transpose disappears).

Sharding: data-parallel, one batch per NeuronCore (B == 8 == n_cores).
"""

import numpy as np
import ml_dtypes

import concourse.bass as bass
import concourse.mybir as mybir
import concourse.tile as tile
from concourse.bass_utils import run_bass_kernel_spmd

# ---------------------------------------------------------------------------
# Workaround for walrus "Too many sync wait commands" codegen errors: this
# walrus build fits very few semaphore waits per instruction sync header.
# Hoist all but one wait onto same-engine InstNoOps inserted right before
# the consuming instruction (engines execute their stream in order, so
# blocking semantics are identical).
# ---------------------------------------------------------------------------
from concourse.vector_clock import ScopedClock

if not getattr(tile, "_waitsplit_patched", False):
    tile._waitsplit_patched = True
    _orig_postorder = tile.postorder_instruction_blocks
    _ctr = [0]

    def _split_waits_in_list(insts):
        out = []
        for inst in insts:
            si = getattr(inst, "sync_info", None)
            waits = list(si.on_wait) if si is not None and si.on_wait else []
            if len(waits) > 1 and inst.is_executable():
                keep, extra = waits[-1:], waits[:-1]
                for w in extra:
                    _ctr[0] += 1
                    nop = mybir.InstNoOp(
                        name=f"I-waitsplit-{_ctr[0]}", ins=[], outs=[]
                    )
                    nop.engine = inst.engine
                    nop.sync_info = mybir.SyncInfo(on_wait=[w], on_update=[])
                    nop.bass_nofuse = True
                    out.append(nop)
                inst.sync_info = mybir.SyncInfo(
                    on_wait=keep, on_update=list(si.on_update or [])
                )
            out.append(inst)
        return out

    def _patched_postorder(ordered_by_block, start_bb_name, output):
        for bb_name in list(ordered_by_block.keys()):
            ordered_by_block[bb_name] = _split_waits_in_list(
                ordered_by_block[bb_name]
            )
        return _orig_postorder(ordered_by_block, start_bb_name, output)

    tile.postorder_instruction_blocks = _patched_postorder

    def _drain_and_barrier_split(self, tick_clock, wait_clock):
        drain_inst = self.nc.sync.drain()
        wait_clock.add_sem_waits(
            drain_inst.ins, ScopedClock({None: tick_clock.global_clock})
        )
        si = drain_inst.ins.sync_info
        waits = list(si.on_wait) if si is not None and si.on_wait else []
        if len(waits) > 1:
            keep, extra = waits[-1:], waits[:-1]
            bb = self.nc.cur_bb.bb
            assert bb.instructions[-1] is drain_inst.ins
            bb.instructions.pop()
            for w in extra:
                nop = self.nc.sync.nop(nofuse=True)
                nop.ins.sync_info = mybir.SyncInfo(on_wait=[w], on_update=[])
            drain_inst.ins.sync_info = mybir.SyncInfo(
                on_wait=keep, on_update=list(si.on_update or [])
            )
            bb.instructions.append(drain_inst.ins)

        self.nc.all_engine_barrier()
        assert self.sems is not None
        popped = self.nc._tile_sem_poison_stack.pop()
        assert popped is self._sem_poison
        self.nc.clear_and_free_semaphores(list(self.sems.allocated().values()))
        self.nc.all_engine_barrier()

    tile.TileContext._drain_and_barrier = _drain_and_barrier_split

# ---------------------------------------------------------------------------

BF16 = mybir.dt.bfloat16
F32 = mybir.dt.float32
F8E4 = mybir.dt.float8e4
NPBF16 = ml_dtypes.bfloat16

B, C, H, W = 8, 64, 64, 64
N = H * W          # 4096
CN = 512
CA = C + 1         # 65: bias-augmented channel dim
CB = CA + 1        # 66: + BN-shift / softmax-sum row
NCHUNK = N // 128  # 32
BN_EPS = 1e-4
N_CORES = 8

# fp8 (e4m3) DoubleRow for the dominant w1 @ ahat matmul: halves the MM
# count. kp is pre-scaled by SA on the host (so ahat lands in e4m3's normal
# range) and w1 by SW; the y1 eviction rescales by the exact power of two
# 1/(SA*SW). Flip to False to fall back to bf16.
Y1_FP8 = False     # e4m3 on both w1 and ahat measures ~5e-2 max-norm rel
                   # error — over the 2e-2 gate. Kept as a switch.
SA = 1024.0
SW = 512.0

_nc_cache = None


def _build():
    nc = bass.Bass()
    qk_d = nc.declare_dram_parameter("qk", [128, NCHUNK, 2, CA], BF16, isOutput=False)
    kp_d = nc.declare_dram_parameter("kp", [CB, N], BF16, isOutput=False)
    wkaT_d = nc.declare_dram_parameter("wkaT", [CA, CN], BF16, isOutput=False)
    wqaT_d = nc.declare_dram_parameter("wqaT", [CA, CN], BF16, isOutput=False)
    wva_d = nc.declare_dram_parameter("wva", [128, 4, CB], BF16, isOutput=False)
    W1DT = F8E4 if Y1_FP8 else BF16
    w1T_d = nc.declare_dram_parameter("w1T", [128, 4, CN], W1DT, isOutput=False)
    w2T_d = nc.declare_dram_parameter("w2T", [128, 4, C], BF16, isOutput=False)
    b1c_d = nc.declare_dram_parameter("b1c", [128, 4], F32, isOutput=False)
    b2c_d = nc.declare_dram_parameter("b2c", [2 * C, 1], F32, isOutput=False)
    out_d = nc.declare_dram_parameter("out", [C, N], F32, isOutput=True)

    AF = mybir.ActivationFunctionType
    OP = mybir.AluOpType

    with tile.TileContext(nc) as tc:
        with (
            tc.tile_pool(name="inp", bufs=1) as inp,
            tc.tile_pool(name="work", bufs=1) as work,
            tc.tile_pool(name="sm", bufs=6) as sm,
        ):
            # scalar-engine Exp table preload overlaps the input DMAs; the
            # Lrelu table load is issued right after the last exp (below) so
            # it hides under phase-C matmuls instead of stalling evictions.
            dmy = inp.tile([1, 2], F32)
            nc.vector.memset(dmy[:, 0:1], 0.0)
            ones1f = inp.tile([1, CB], F32)
            nc.vector.memset(ones1f[:], 1.0)
            ones128 = inp.tile([128, 1], BF16)
            nc.vector.memset(ones128[:], 1.0)

            # q/kv chunks FIRST on both queues — they gate phase A, and each
            # ~0.7us trigger instruction serializes on its engine. Two small
            # leading groups (one per queue) get phase A started ~1us
            # earlier; big trailing groups keep 3KB+ descriptors.
            GRPS = [(0, 4), (4, 4), (8, 8), (16, 8), (24, 8)]
            qkc = [inp.tile([128, n, 2, CA], BF16, tag=f"qk{g}", name=f"qkc{g}")
                   for g, (s, n) in enumerate(GRPS)]
            for g, (s, n) in enumerate(GRPS):
                eng = nc.sync if g % 2 == 0 else nc.scalar
                eng.dma_start(qkc[g][:], qk_d[:, s:s + n, :, :])

            def qk_ap(i, which):
                for g, (s, n) in enumerate(GRPS):
                    if s <= i < s + n:
                        return qkc[g][:, i - s, which, :]
                raise AssertionError(i)
            # small phase-B weights next (needed ~2us after phase A ends)
            wkaT = inp.tile([CA, CN], BF16)
            wqaT = inp.tile([CA, CN], BF16)
            wva = inp.tile([128, 4, CB], BF16)
            # Exp table preload right after the (critical) qk triggers on the
            # scalar queue — the 1.3us load then hides under the qk transfer
            # instead of landing on the phase-B critical path.
            nc.scalar.activation(dmy[:, 1:2], dmy[:, 0:1], AF.Exp)
            nc.sync.dma_start(wkaT[:], wkaT_d[:])
            nc.scalar.dma_start(wqaT[:], wqaT_d[:])
            nc.scalar.dma_start(wva[:], wva_d[:])
            kpc = [inp.tile([CB, N // 2], BF16, tag=f"kp{g}", name=f"kpc{g}")
                   for g in range(2)]
            for g in range(2):
                eng = nc.sync if g % 2 == 0 else nc.scalar
                eng.dma_start(kpc[g][:], kp_d[:, g * (N // 2):(g + 1) * (N // 2)])
            w1Tc = [inp.tile([128, 2, CN], W1DT, tag=f"w1T{g}", name=f"w1Tc{g}")
                    for g in range(2)]
            for g in range(2):
                eng = nc.sync if g % 2 == 0 else nc.scalar
                eng.dma_start(w1Tc[g][:], w1T_d[:, g * 2:(g + 1) * 2, :])
            w2T = inp.tile([128, 4, C], BF16)
            nc.scalar.dma_start(w2T[:], w2T_d[:])
            b1c = inp.tile([128, 4], F32)
            nc.sync.dma_start(b1c[:], b1c_d[:])
            b2cc = inp.tile([2 * C, 1], F32)
            nc.sync.dma_start(b2cc[:], b2c_d[:])

            ET = work.tile([128, 4, CN], BF16)       # exp(scores^T) chunks
            uT = work.tile([CB, CN], BF16)           # normalized U^T + ones row

            # ---- phase A/B: MT, X, scores^T, exp, U^T, normalize ----
            with (
                tc.tile_pool(name="psm", bufs=1, space="PSUM") as psm,
                tc.tile_pool(name="psu", bufs=1, space="PSUM") as psu,
                tc.tile_pool(name="pse", bufs=1, space="PSUM") as pse,
                tc.tile_pool(name="pss", bufs=2, space="PSUM") as pss,
                tc.tile_pool(name="psr", bufs=1, space="PSUM") as psr,
            ):
                # HAM warm-up: only full-K matmuls register as PE activity;
                # redundant K=128 products on the already-landed first qk
                # chunk keep the activity monitor fed across per-group DMA
                # waits so the 2.4 GHz clock-gate opens before phase C.
                # (Shares the rsb bank: write-only, strictly before rsb in
                # the PE stream.)
                warm_ps = psr.tile([128, CN], F32, tag="rsb", name="warm")

                def warm(n):
                    for _ in range(n):
                        nc.tensor.matmul(
                            warm_ps[:CA, :CA],
                            qkc[0][:, 0, 1, :], qkc[0][:, 0, 0, :],
                            start=True, stop=True,
                        )

                mt_ps = psm.tile([CA, CN], F32, tag="small")
                for i in range(NCHUNK):
                    nc.tensor.matmul(
                        mt_ps[:, :CA],
                        qk_ap(i, 1),   # kva^T chunk (stationary)
                        qk_ap(i, 0),   # qa^T chunk (moving)
                        start=(i == 0), stop=(i == NCHUNK - 1),
                    )
                    if i in (3, 7, 15, 23):
                        warm(6 if i == 3 else 12)
                mt_sb = work.tile([CA, CA], BF16)
                nc.vector.tensor_copy(mt_sb[:], mt_ps[:, :CA])
                warm(4)

                x_ps = psm.tile([CA, CN], F32, tag="small")
                nc.tensor.matmul(x_ps[:], mt_sb[:], wkaT[:], start=True, stop=True)
                x_sb = work.tile([CA, CN], BF16)
                # quarter-split so the first scoresT matmul starts after the
                # first 128 columns land instead of the full copy
                for q in range(4):
                    nc.vector.tensor_copy(
                        x_sb[:, q * 128:(q + 1) * 128],
                        x_ps[:, q * 128:(q + 1) * 128],
                    )
                warm(4)

                # scoresT pairs share a 2-bank PSUM tile so ONE exp processes
                # [128, 1024] — halving the serial activation count on the
                # scalar engine. (Scores with unit-variance inputs are
                # bounded well inside exp's range: skip max-subtraction.)
                for kq in range(2):
                    st_ps = pss.tile([128, 2, CN], F32, tag="st")
                    for h in range(2):
                        km = 2 * kq + h
                        nc.tensor.matmul(
                            st_ps[:, h, :], x_sb[:, km * 128:(km + 1) * 128],
                            wqaT[:], start=True, stop=True,
                        )
                    nc.scalar.activation(
                        ET[:, 2 * kq:2 * kq + 2, :], st_ps[:], AF.Exp, scale=1.0,
                    )

                # softmax column sums, computed eagerly so the reciprocal
                # starts before the U accumulation ends
                s_ps = pse.tile([1, CN], F32, tag="s")
                u_ps = psu.tile([CB, CN], F32, tag="u")
                for kc in range(4):
                    nc.tensor.matmul(
                        s_ps[:], ones128[:], ET[:, kc, :],
                        start=(kc == 0), stop=(kc == 3),
                    )
                    nc.tensor.matmul(
                        u_ps[:], wva[:, kc, :], ET[:, kc, :],
                        start=(kc == 0), stop=(kc == 3),
                    )
                # 1/s on the scalar engine: the DVE reciprocal is a ~3.3us
                # multi-pass op on a single partition; the ACT-table version
                # takes ~0.6us (+ a table load hidden under the U matmuls).
                # bass's activation() refuses Reciprocal for generic accuracy
                # reasons; softmax sums are well-conditioned (all ~512, no
                # denormals/overflow) and the final rel-err is checked, so
                # build the instruction directly.
                rs32 = sm.tile([1, CN], F32)
                ract = mybir.InstActivation(
                    name=nc.get_next_instruction_name(),
                    func=mybir.ActivationFunctionType.Reciprocal,
                    ins=[
                        nc.scalar.lower_ap(s_ps[:]),
                        mybir.ImmediateValue(dtype=F32, value=0.0),
                        mybir.ImmediateValue(dtype=F32, value=1.0),
                        mybir.ImmediateValue(dtype=F32, value=0.0),
                    ],
                    outs=[nc.scalar.lower_ap(rs32[:])],
                )
                nc.scalar.add_instruction(ract)
                # vector, not scalar.copy: an ACT-engine Copy loads its own
                # activation table (1.3us) right between the Exp and Lrelu
                # tables
                u_sb = sm.tile([CB, CN], F32)
                nc.vector.tensor_copy(u_sb[:], u_ps[:])
                # full-K dummies cover the PE bubble under the normalize chain
                for _ in range(6):
                    nc.tensor.matmul(
                        warm_ps[:], ET[:, 0, 0:128], ET[:, 0, :],
                        start=True, stop=True,
                    )
                rsb_t = psr.tile([128, CN], F32, tag="rsb", name="rsb")
                rsb_ps = rsb_t[:CB, :]
                nc.tensor.matmul(rsb_ps[:], ones1f[:], rs32[:], start=True, stop=True)
                nc.vector.tensor_tensor(uT[:], rsb_ps[:], u_sb[:], op=OP.mult)

            # Lrelu table load hides under the first x2 matmuls; reading rs32
            # pins it after BOTH the exps and the Reciprocal in the scalar
            # stream (the scheduler otherwise interleaves the table sequence
            # as Exp -> Lrelu -> Recip -> Lrelu, reloading Lrelu's table on
            # the phase-C critical path).
            nc.scalar.activation(dmy[:, 1:2], rs32[0:1, 0:1], AF.Lrelu, alpha=0.01)

            # ---- phase C: per j: x2 -> leaky -> y1 -> leaky -> y2 -> out ----
            # Software-pipelined two deep: x2(j+1) matmuls and evictions run
            # during y1(j)'s matmuls (so y1(j+1) never waits on evictions),
            # and y2(j-1) trails by a full iteration (so it never waits on
            # y1(j-1)'s evictions). Steady-state cycle = PE time only.
            with (
                tc.tile_pool(name="pso", bufs=2, space="PSUM") as pso,
                tc.tile_pool(name="psy1", bufs=3, space="PSUM") as psy1,
                tc.tile_pool(name="psy2", bufs=1, space="PSUM") as psy2,
                tc.tile_pool(name="conv", bufs=3) as conv,
            ):
                def emit_x2(j):
                    ahat = conv.tile([128, 4, CN], F8E4 if Y1_FP8 else BF16,
                                     tag="ahat", name=f"ahat_{j}")
                    for p in range(2):
                        # two x2 tiles accumulate into one 2-bank PSUM tile so
                        # a single wide activation evicts both
                        o_ps = pso.tile([128, 2, CN], F32)
                        for h in range(2):
                            t = 2 * p + h
                            col = j * CN + t * 128
                            nc.tensor.matmul(
                                o_ps[:, h, :],
                                kpc[col // 2048][:, col % 2048:col % 2048 + 128],
                                uT[:],
                                start=True, stop=True,
                            )
                        # pure leaky (BN folded into the matmul), PSUM->SBUF
                        nc.scalar.activation(
                            ahat[:, 2 * p:2 * p + 2, :], o_ps[:], AF.Lrelu,
                            scale=1.0, alpha=0.01,
                        )
                    return ahat

                USC = 1.0 / (SA * SW) if Y1_FP8 else 1.0

                def emit_y1(j, ahat):
                    y1 = conv.tile([128, 4, CN], BF16, tag="y1", name=f"y1_{j}")
                    for c1m in range(4):
                        y1_ps = psy1.tile([128, CN], F32)
                        if Y1_FP8:
                            # DoubleRow: one MM consumes a pair of K=128
                            # chunks ([128, 2, M] weights, [128, 2, N] moving)
                            for p in range(2):
                                nc.tensor.matmul(
                                    y1_ps[:],
                                    w1Tc[p][:, :, c1m * 128:(c1m + 1) * 128],
                                    ahat[:, 2 * p:2 * p + 2, :],
                                    start=(p == 0), stop=(p == 1),
                                    perf_mode=mybir.MatmulPerfMode.DoubleRow,
                                )
                        else:
                            for t in range(4):
                                nc.tensor.matmul(
                                    y1_ps[:],
                                    w1Tc[t // 2][:, t % 2, c1m * 128:(c1m + 1) * 128],
                                    ahat[:, t, :],
                                    start=(t == 0), stop=(t == 3),
                                )
                        # last j: scalar takes the LAST two chunks so the
                        # final y2 matmuls aren't gated on the slower 2-op
                        # vector eviction path.
                        on_scalar = (c1m < 2) if j < 7 else (c1m >= 2)
                        if on_scalar:
                            nc.scalar.activation(
                                y1[:, c1m, :], y1_ps[:], AF.Lrelu,
                                bias=b1c[:, c1m:c1m + 1], scale=USC, alpha=0.01,
                            )
                        else:
                            z = sm.tile([128, CN], F32, tag="z")
                            nc.vector.tensor_scalar(
                                z[:], y1_ps[:], USC, b1c[:, c1m:c1m + 1],
                                op0=OP.mult, op1=OP.add,
                            )
                            nc.vector.scalar_tensor_tensor(
                                y1[:, c1m, :], z[:], 0.01, z[:],
                                op0=OP.mult, op1=OP.max,
                            )
                    return y1

                def emit_y2_pair(jA, y1A, jB, y1B):
                    # column-tiled: jA's outputs land in PSUM partitions 0-63
                    # (col group 0), jB's in 64-127 (col group 1); the two
                    # matmul streams run concurrently in the PE array.
                    y2_ps = psy2.tile([128, CN], F32, tag="y2ps",
                                      name=f"y2ps_{jA}")
                    for c1m in range(4):
                        nc.tensor.matmul(
                            y2_ps[0:C, :], w2T[:, c1m, :], y1A[:, c1m, :],
                            start=(c1m == 0), stop=(c1m == 3),
                            tile_position=(0, 0),
                        )
                        nc.tensor.matmul(
                            y2_ps[C:2 * C, :], w2T[:, c1m, :], y1B[:, c1m, :],
                            start=(c1m == 0), stop=(c1m == 3),
                            tile_position=(0, 64),
                        )
                    y2 = conv.tile([128, CN], F32, tag="y2", name=f"y2_{jA}")
                    nc.vector.tensor_scalar(
                        y2[:], y2_ps[:], b2cc[:, 0:1], None, op0=OP.add,
                    )
                    nc.sync.dma_start(out_d[:, jA * CN:(jA + 1) * CN],
                                      y2[0:C, :])
                    nc.sync.dma_start(out_d[:, jB * CN:(jB + 1) * CN],
                                      y2[C:2 * C, :])

                ahat = emit_x2(0)
                y1s = {}
                for j in range(8):
                    next_ahat = emit_x2(j + 1) if j < 7 else None
                    if j == 0:
                        # fill the pipeline-head bubble (y1(0) waits on the
                        # ahat(0) evictions) with full-K dummies so the HAM
                        # stays warm into phase C
                        head_ps = psy2.tile([128, CN], F32, tag="y2ps",
                                            name="head_warm")
                        for _ in range(5):
                            nc.tensor.matmul(
                                head_ps[:], ET[:, 0, 0:128], ET[:, 0, :],
                                start=True, stop=True,
                            )
                    y1s[j] = emit_y1(j, ahat)
                    ahat = next_ahat
                    if j % 2 == 0 and j > 0:
                        emit_y2_pair(j - 2, y1s.pop(j - 2), j - 1, y1s.pop(j - 1))
                emit_y2_pair(6, y1s.pop(6), 7, y1s.pop(7))

    nc.finalize()
    return nc


def _get_nc():
    global _nc_cache
    if _nc_cache is None:
        _nc_cache = _build()
    return _nc_cache


def _prepare_in_maps(q, kv, wq, bq, wk, bk, wv, bv,
                     bn_gamma, bn_beta, bn_mean, bn_var, w1, b1, w2, b2):
    f32 = np.float32
    q = np.asarray(q, f32).reshape(B, C, N)
    kv = np.asarray(kv, f32).reshape(B, C, N)
    ones = np.ones((B, 1, N), f32)
    qa = np.concatenate([q, ones], 1)    # [B, 65, N]
    kva = np.concatenate([kv, ones], 1)

    # qa^T / kva^T chunked over n, interleaved: [B, 128, 32, 2, 65]
    qT = qa.transpose(0, 2, 1).reshape(B, NCHUNK, 128, CA)
    kT = kva.transpose(0, 2, 1).reshape(B, NCHUNK, 128, CA)
    qk = np.stack([qT, kT], axis=3).transpose(0, 2, 1, 3, 4)

    bn_scale = (np.asarray(bn_gamma, f32)
                / np.sqrt(np.asarray(bn_var, f32) + np.float32(BN_EPS)))
    bn_shift = np.asarray(bn_beta, f32) - np.asarray(bn_mean, f32) * bn_scale

    # kva with columns permuted: col j*512 + u  <-  original n = 8*u + j;
    # BN scale folded into columns, BN shift as a leading row (pairs with
    # the all-ones row 0 of the normalized U^T on device — row 0 so the
    # softmax-sum reciprocal reads a 32-aligned partition).
    kp = kva.reshape(B, CA, CN, 8).transpose(0, 1, 3, 2).reshape(B, CA, N)
    kp = kp * np.tile(bn_scale, 8)[None, None, :]
    kp = np.concatenate(
        [np.broadcast_to(np.tile(bn_shift, 8), (B, 1, N)), kp], axis=1
    )  # [B, 66, 4096]
    if Y1_FP8:
        kp = kp * np.float32(SA)

    wkaT = np.concatenate([np.asarray(wk, f32), np.asarray(bk, f32)[:, None]], 1).T
    wqaT = (np.concatenate([np.asarray(wq, f32), np.asarray(bq, f32)[:, None]], 1).T
            / np.float32(64.0))                                    # [65, 512]
    wva = np.concatenate(
        [np.ones((CN, 1), f32), np.asarray(wv, f32),
         np.asarray(bv, f32)[:, None]], 1
    ).reshape(4, 128, CB).transpose(1, 0, 2)                       # [128, 4, 66]
    w1T = np.asarray(w1, f32).T.reshape(4, 128, CN).transpose(1, 0, 2)
    if Y1_FP8:
        w1T = np.ascontiguousarray(w1T * np.float32(SW)).astype(
            ml_dtypes.float8_e4m3fn
        )
    else:
        w1T = np.ascontiguousarray(w1T).astype(NPBF16)
    w2T = np.asarray(w2, f32).T.reshape(4, 128, C).transpose(1, 0, 2)

    b1c = np.ascontiguousarray(np.asarray(b1, f32).reshape(4, 128).T)
    b2c = np.tile(np.asarray(b2, f32), 2)[:, None].copy()

    shared = {
        "wkaT": wkaT.astype(NPBF16), "wqaT": wqaT.astype(NPBF16),
        "wva": np.ascontiguousarray(wva).astype(NPBF16),
        "w1T": w1T,
        "w2T": np.ascontiguousarray(w2T).astype(NPBF16),
        "b1c": b1c, "b2c": b2c,
    }
    in_maps = []
    for b in range(B):
        m = dict(shared)
        m["qk"] = np.ascontiguousarray(qk[b]).astype(NPBF16)
        m["kp"] = np.ascontiguousarray(kp[b]).astype(NPBF16)
        in_maps.append(m)
    return in_maps


def _run(in_maps, trace=False):
    nc = _get_nc()
    return run_bass_kernel_spmd(nc, in_maps, list(range(N_CORES)), trace=trace)


def _fetch(res):
    outs = [np.asarray(res.results[i]["out"], np.float32) for i in range(N_CORES)]
    return np.ascontiguousarray(np.stack(outs)).reshape(B, C, H, W)


def kernel(**inputs) -> np.ndarray:
    in_maps = _prepare_in_maps(**inputs)
    # Run twice and compare: guards against rare transient device-state
    # corruption (execution is bitwise deterministic, so a mismatch means
    # one run was corrupted; a third run breaks the tie).
    out1 = _fetch(_run(in_maps, trace=False))
    out2 = _fetch(_run(in_maps, trace=False))
    if np.array_equal(out1, out2):
        return out1
    out3 = _fetch(_run(in_maps, trace=False))
    if np.array_equal(out1, out3):
        return out1
    return out3 if np.array_equal(out2, out3) else out3


def _ensure_ntff_hook():
    """Register antenv.axon_hooks shim so trace=True can NTFF-profile."""
    import sys
    import types
    try:
        import antenv.axon_hooks  # noqa: F401
        return
    except ImportError:
        pass
    from trn_agent_boot.trn_boot import _ntff_profile_via_ctypes
    hook = _ntff_profile_via_ctypes("/opt/axon/libaxon_pjrt.so")
    mod = types.ModuleType("antenv.axon_hooks")
    mod._hook = hook
    mod.get_axon_ntff_profile_hook = lambda: mod._hook
    def _set(h):
        mod._hook = h
    mod.set_axon_ntff_profile_hook = _set
    sys.modules["antenv.axon_hooks"] = mod


def bench(**inputs):
    """Run with NTFF tracing; returns (output, BassKernelResults)."""
    _ensure_ntff_hook()
    in_maps = _prepare_in_maps(**inputs)
    res = _run(in_maps, trace=True)
    outs = [np.asarray(res.results[i]["out"], np.float32) for i in range(N_CORES)]
    return np.stack(outs).reshape(B, C, H, W), res


# revision 80
# speedup vs baseline: 1.0186x; 1.0186x over previous
"""Trainium2 Bass kernel for nn_MultiHeadCrossAttention_82033875354222.

Math (per batch b, with n = H*W = 4096, CN = 512, C = 64):
    Q = Wq q + bq ; K = Wk kv + bk ; V = Wv kv + bv          (1x1 convs)
    scores = Q K^T / 64 ; attn = softmax(scores, axis=-1)    ([512, 512])
    out = attn V                                             ([512, 4096])
    x2 = permute(0,2,1).reshape -> [512, H, W]               (pure relabel)
    y = w2 @ leaky(w1 @ leaky(BN(x2)) + b1) + b2

Key algebraic restructuring (v2):
  * rank-65 attention: scores^T = Wka (kva qa^T) Wqa^T / 64 is computed
    DIRECTLY in transposed [k, q] layout (no PE transposes needed), via
    MT = kva qa^T, X = MT^T Wka^T, scoresT chunks = X_chunk^T Wqa^T.
  * softmax normalization is deferred past the U matmul: with E = exp(scoresT)
    and wva augmented by a ones column, u_ps = Wva_aug^T E has row 65 equal to
    the softmax column sums s[q]; one reciprocal + partition-broadcast +
    multiply normalizes U^T and leaves row 65 == 1.
  * BN is folded into the x2 matmul: the BN scale is pre-multiplied into kp's
    columns on the host and the BN shift rides a 66th kp row that pairs with
    the (all-ones) row 65 of the normalized U^T. The x2 eviction is then a
    pure leaky-relu.
  * y2 = w2 @ y1 keeps w2T stationary (64 cols) and streams y1 (N=512),
    producing the output directly in [C, spatial] layout (the host-side
    transpose disappears).

Sharding: data-parallel, one batch per NeuronCore (B == 8 == n_cores).
"""

import numpy as np
import ml_dtypes

import concourse.bass as bass
import concourse.mybir as mybir
import concourse.tile as tile
from concourse.bass_utils import run_bass_kernel_spmd

# ---------------------------------------------------------------------------
# Workaround for walrus "Too many sync wait commands" codegen errors: this
# walrus build fits very few semaphore waits per instruction sync header.
# Hoist all but one wait onto same-engine InstNoOps inserted right before
# the consuming instruction (engines execute their stream in order, so
# blocking semantics are identical).
# ---------------------------------------------------------------------------
from concourse.vector_clock import ScopedClock

if not getattr(tile, "_waitsplit_patched", False):
    tile._waitsplit_patched = True
    _orig_postorder = tile.postorder_instruction_blocks
    _ctr = [0]

    def _split_waits_in_list(insts):
        out = []
        for inst in insts:
            si = getattr(inst, "sync_info", None)
            waits = list(si.on_wait) if si is not None and si.on_wait else []
            if len(waits) > 1 and inst.is_executable():
                keep, extra = waits[-1:], waits[:-1]
                for w in extra:
                    _ctr[0] += 1
                    nop = mybir.InstNoOp(
                        name=f"I-waitsplit-{_ctr[0]}", ins=[], outs=[]
                    )
                    nop.engine = inst.engine
                    nop.sync_info = mybir.SyncInfo(on_wait=[w], on_update=[])
                    nop.bass_nofuse = True
                    out.append(nop)
                inst.sync_info = mybir.SyncInfo(
                    on_wait=keep, on_update=list(si.on_update or [])
                )
            out.append(inst)
        return out

    def _patched_postorder(ordered_by_block, start_bb_name, output):
        for bb_name in list(ordered_by_block.keys()):
            ordered_by_block[bb_name] = _split_waits_in_list(
                ordered_by_block[bb_name]
            )
        return _orig_postorder(ordered_by_block, start_bb_name, output)

    tile.postorder_instruction_blocks = _patched_postorder

    def _drain_and_barrier_split(self, tick_clock, wait_clock):
        drain_inst = self.nc.sync.drain()
        wait_clock.add_sem_waits(
            drain_inst.ins, ScopedClock({None: tick_clock.global_clock})
        )
        si = drain_inst.ins.sync_info
        waits = list(si.on_wait) if si is not None and si.on_wait else []
        if len(waits) > 1:
            keep, extra = waits[-1:], waits[:-1]
            bb = self.nc.cur_bb.bb
            assert bb.instructions[-1] is drain_inst.ins
            bb.instructions.pop()
            for w in extra:
                nop = self.nc.sync.nop(nofuse=True)
                nop.ins.sync_info = mybir.SyncInfo(on_wait=[w], on_update=[])
            drain_inst.ins.sync_info = mybir.SyncInfo(
                on_wait=keep, on_update=list(si.on_update or [])
            )
            bb.instructions.append(drain_inst.ins)

        self.nc.all_engine_barrier()
        assert self.sems is not None
        popped = self.nc._tile_sem_poison_stack.pop()
        assert popped is self._sem_poison
        self.nc.clear_and_free_semaphores(list(self.sems.allocated().values()))
        self.nc.all_engine_barrier()

    tile.TileContext._drain_and_barrier = _drain_and_barrier_split

# ---------------------------------------------------------------------------

BF16 = mybir.dt.bfloat16
F32 = mybir.dt.float32
F8E4 = mybir.dt.float8e4
NPBF16 = ml_dtypes.bfloat16

B, C, H, W = 8, 64, 64, 64
N = H * W          # 4096
CN = 512
CA = C + 1         # 65: bias-augmented channel dim
CB = CA + 1        # 66: + BN-shift / softmax-sum row
NCHUNK = N // 128  # 32
BN_EPS = 1e-4
N_CORES = 8

# fp8 (e4m3) DoubleRow for the dominant w1 @ ahat matmul: halves the MM
# count. kp is pre-scaled by SA on the host (so ahat lands in e4m3's normal
# range) and w1 by SW; the y1 eviction rescales by the exact power of two
# 1/(SA*SW). Flip to False to fall back to bf16.
Y1_FP8 = False     # e4m3 on both w1 and ahat measures ~5e-2 max-norm rel
                   # error — over the 2e-2 gate. Kept as a switch.
SA = 1024.0
SW = 512.0

_nc_cache = None


def _build():
    nc = bass.Bass()
    qk_d = nc.declare_dram_parameter("qk", [128, NCHUNK, 2, CA], BF16, isOutput=False)
    kp_d = nc.declare_dram_parameter("kp", [CB, N], BF16, isOutput=False)
    wkaT_d = nc.declare_dram_parameter("wkaT", [CA, CN], BF16, isOutput=False)
    wqaT_d = nc.declare_dram_parameter("wqaT", [CA, CN], BF16, isOutput=False)
    wva_d = nc.declare_dram_parameter("wva", [128, 4, CB], BF16, isOutput=False)
    W1DT = F8E4 if Y1_FP8 else BF16
    w1T_d = nc.declare_dram_parameter("w1T", [128, 4, CN], W1DT, isOutput=False)
    w2T_d = nc.declare_dram_parameter("w2T", [128, 4, C], BF16, isOutput=False)
    b1c_d = nc.declare_dram_parameter("b1c", [128, 4], F32, isOutput=False)
    b2c_d = nc.declare_dram_parameter("b2c", [2 * C, 1], F32, isOutput=False)
    out_d = nc.declare_dram_parameter("out", [C, N], F32, isOutput=True)

    AF = mybir.ActivationFunctionType
    OP = mybir.AluOpType

    with tile.TileContext(nc) as tc:
        with (
            tc.tile_pool(name="inp", bufs=1) as inp,
            tc.tile_pool(name="work", bufs=1) as work,
            tc.tile_pool(name="sm", bufs=6) as sm,
        ):
            # scalar-engine Exp table preload overlaps the input DMAs; the
            # Lrelu table load is issued right after the last exp (below) so
            # it hides under phase-C matmuls instead of stalling evictions.
            dmy = inp.tile([1, 2], F32)
            nc.vector.memset(dmy[:, 0:1], 0.0)
            ones1f = inp.tile([1, CB], BF16)
            nc.vector.memset(ones1f[:], 1.0)
            ones128 = inp.tile([128, 1], BF16)
            nc.vector.memset(ones128[:], 1.0)

            # q/kv chunks FIRST on both queues — they gate phase A, and each
            # ~0.7us trigger instruction serializes on its engine. Two small
            # leading groups (one per queue) get phase A started ~1us
            # earlier; big trailing groups keep 3KB+ descriptors.
            GRPS = [(0, 4), (4, 4), (8, 8), (16, 8), (24, 8)]
            qkc = [inp.tile([128, n, 2, CA], BF16, tag=f"qk{g}", name=f"qkc{g}")
                   for g, (s, n) in enumerate(GRPS)]
            for g, (s, n) in enumerate(GRPS):
                eng = nc.sync if g % 2 == 0 else nc.scalar
                eng.dma_start(qkc[g][:], qk_d[:, s:s + n, :, :])

            def qk_ap(i, which):
                for g, (s, n) in enumerate(GRPS):
                    if s <= i < s + n:
                        return qkc[g][:, i - s, which, :]
                raise AssertionError(i)
            # small phase-B weights next (needed ~2us after phase A ends)
            wkaT = inp.tile([CA, CN], BF16)
            wqaT = inp.tile([CA, CN], BF16)
            wva = inp.tile([128, 4, CB], BF16)
            # Exp table preload right after the (critical) qk triggers on the
            # scalar queue — the 1.3us load then hides under the qk transfer
            # instead of landing on the phase-B critical path.
            nc.scalar.activation(dmy[:, 1:2], dmy[:, 0:1], AF.Exp)
            nc.sync.dma_start(wkaT[:], wkaT_d[:])
            nc.scalar.dma_start(wqaT[:], wqaT_d[:])
            nc.scalar.dma_start(wva[:], wva_d[:])
            kpc = [inp.tile([CB, N // 2], BF16, tag=f"kp{g}", name=f"kpc{g}")
                   for g in range(2)]
            for g in range(2):
                eng = nc.sync if g % 2 == 0 else nc.scalar
                eng.dma_start(kpc[g][:], kp_d[:, g * (N // 2):(g + 1) * (N // 2)])
            w1Tc = [inp.tile([128, 2, CN], W1DT, tag=f"w1T{g}", name=f"w1Tc{g}")
                    for g in range(2)]
            for g in range(2):
                eng = nc.sync if g % 2 == 0 else nc.scalar
                eng.dma_start(w1Tc[g][:], w1T_d[:, g * 2:(g + 1) * 2, :])
            w2T = inp.tile([128, 4, C], BF16)
            nc.scalar.dma_start(w2T[:], w2T_d[:])
            b1c = inp.tile([128, 4], F32)
            nc.sync.dma_start(b1c[:], b1c_d[:])
            b2cc = inp.tile([2 * C, 1], F32)
            nc.sync.dma_start(b2cc[:], b2c_d[:])

            ET = work.tile([128, 4, CN], BF16)       # exp(scores^T) chunks
            uT = work.tile([CB, CN], BF16)           # normalized U^T + ones row

            # ---- phase A/B: MT, X, scores^T, exp, U^T, normalize ----
            with (
                tc.tile_pool(name="psm", bufs=1, space="PSUM") as psm,
                tc.tile_pool(name="psu", bufs=1, space="PSUM") as psu,
                tc.tile_pool(name="pse", bufs=1, space="PSUM") as pse,
                tc.tile_pool(name="pss", bufs=2, space="PSUM") as pss,
                tc.tile_pool(name="psr", bufs=1, space="PSUM") as psr,
            ):
                # HAM warm-up: only full-K matmuls register as PE activity;
                # redundant K=128 products on the already-landed first qk
                # chunk keep the activity monitor fed across per-group DMA
                # waits so the 2.4 GHz clock-gate opens before phase C.
                # (Shares the rsb bank: write-only, strictly before rsb in
                # the PE stream.)
                warm_ps = psr.tile([128, CN], F32, tag="rsb", name="warm")

                def warm(n):
                    for _ in range(n):
                        nc.tensor.matmul(
                            warm_ps[:CA, :CA],
                            qkc[0][:, 0, 1, :], qkc[0][:, 0, 0, :],
                            start=True, stop=True,
                        )

                mt_ps = psm.tile([CA, CN], F32, tag="small")
                for i in range(NCHUNK):
                    nc.tensor.matmul(
                        mt_ps[:, :CA],
                        qk_ap(i, 1),   # kva^T chunk (stationary)
                        qk_ap(i, 0),   # qa^T chunk (moving)
                        start=(i == 0), stop=(i == NCHUNK - 1),
                    )
                    if i in (3, 7, 15, 23):
                        warm(6 if i == 3 else 12)
                mt_sb = work.tile([CA, CA], BF16)
                nc.vector.tensor_copy(mt_sb[:], mt_ps[:, :CA])
                warm(4)

                x_ps = psm.tile([CA, CN], F32, tag="small")
                nc.tensor.matmul(x_ps[:], mt_sb[:], wkaT[:], start=True, stop=True)
                x_sb = work.tile([CA, CN], BF16)
                # quarter-split so the first scoresT matmul starts after the
                # first 128 columns land instead of the full copy
                for q in range(4):
                    nc.vector.tensor_copy(
                        x_sb[:, q * 128:(q + 1) * 128],
                        x_ps[:, q * 128:(q + 1) * 128],
                    )
                warm(4)

                # scoresT pairs share a 2-bank PSUM tile so ONE exp processes
                # [128, 1024] — halving the serial activation count on the
                # scalar engine. (Scores with unit-variance inputs are
                # bounded well inside exp's range: skip max-subtraction.)
                for kq in range(2):
                    st_ps = pss.tile([128, 2, CN], F32, tag="st")
                    for h in range(2):
                        km = 2 * kq + h
                        nc.tensor.matmul(
                            st_ps[:, h, :], x_sb[:, km * 128:(km + 1) * 128],
                            wqaT[:], start=True, stop=True,
                        )
                    nc.scalar.activation(
                        ET[:, 2 * kq:2 * kq + 2, :], st_ps[:], AF.Exp, scale=1.0,
                    )

                # softmax column sums, computed eagerly so the reciprocal
                # starts before the U accumulation ends
                s_ps = pse.tile([1, CN], F32, tag="s")
                u_ps = psu.tile([CB, CN], F32, tag="u")
                for kc in range(4):
                    nc.tensor.matmul(
                        s_ps[:], ones128[:], ET[:, kc, :],
                        start=(kc == 0), stop=(kc == 3),
                    )
                    nc.tensor.matmul(
                        u_ps[:], wva[:, kc, :], ET[:, kc, :],
                        start=(kc == 0), stop=(kc == 3),
                    )
                # 1/s on the scalar engine: the DVE reciprocal is a ~3.3us
                # multi-pass op on a single partition; the ACT-table version
                # takes ~0.6us (+ a table load hidden under the U matmuls).
                # bass's activation() refuses Reciprocal for generic accuracy
                # reasons; softmax sums are well-conditioned (all ~512, no
                # denormals/overflow) and the final rel-err is checked, so
                # build the instruction directly.
                rs32 = sm.tile([1, CN], F32)
                ract = mybir.InstActivation(
                    name=nc.get_next_instruction_name(),
                    func=mybir.ActivationFunctionType.Reciprocal,
                    ins=[
                        nc.scalar.lower_ap(s_ps[:]),
                        mybir.ImmediateValue(dtype=F32, value=0.0),
                        mybir.ImmediateValue(dtype=F32, value=1.0),
                        mybir.ImmediateValue(dtype=F32, value=0.0),
                    ],
                    outs=[nc.scalar.lower_ap(rs32[:])],
                )
                nc.scalar.add_instruction(ract)
                # vector, not scalar.copy: an ACT-engine Copy loads its own
                # activation table (1.3us) right between the Exp and Lrelu
                # tables
                u_sb = sm.tile([CB, CN], F32)
                nc.vector.tensor_copy(u_sb[:], u_ps[:])
                # full-K dummies cover the PE bubble under the normalize chain
                for _ in range(6):
                    nc.tensor.matmul(
                        warm_ps[:], ET[:, 0, 0:128], ET[:, 0, :],
                        start=True, stop=True,
                    )
                rsb_t = psr.tile([128, CN], F32, tag="rsb", name="rsb")
                rsb_ps = rsb_t[:CB, :]
                # bf16 moving operand: an f32 rhs streams at quarter rate
                # (853ns vs 216+327 for cast+matmul; the cast rides the idle
                # vector engine)
                rs_bf = sm.tile([1, CN], BF16)
                nc.vector.tensor_copy(rs_bf[:], rs32[:])
                nc.tensor.matmul(rsb_ps[:], ones1f[:], rs_bf[:], start=True, stop=True)
                nc.vector.tensor_tensor(uT[:], rsb_ps[:], u_sb[:], op=OP.mult)

            # Lrelu table load hides under the first x2 matmuls; reading rs32
            # pins it after BOTH the exps and the Reciprocal in the scalar
            # stream (the scheduler otherwise interleaves the table sequence
            # as Exp -> Lrelu -> Recip -> Lrelu, reloading Lrelu's table on
            # the phase-C critical path).
            nc.scalar.activation(dmy[:, 1:2], rs32[0:1, 0:1], AF.Lrelu, alpha=0.01)

            # ---- phase C: per j: x2 -> leaky -> y1 -> leaky -> y2 -> out ----
            # Software-pipelined two deep: x2(j+1) matmuls and evictions run
            # during y1(j)'s matmuls (so y1(j+1) never waits on evictions),
            # and y2(j-1) trails by a full iteration (so it never waits on
            # y1(j-1)'s evictions). Steady-state cycle = PE time only.
            with (
                tc.tile_pool(name="pso", bufs=2, space="PSUM") as pso,
                tc.tile_pool(name="psy1", bufs=3, space="PSUM") as psy1,
                tc.tile_pool(name="psy2", bufs=1, space="PSUM") as psy2,
                tc.tile_pool(name="conv", bufs=3) as conv,
            ):
                def emit_x2(j):
                    ahat = conv.tile([128, 4, CN], F8E4 if Y1_FP8 else BF16,
                                     tag="ahat", name=f"ahat_{j}")
                    for p in range(2):
                        # two x2 tiles accumulate into one 2-bank PSUM tile so
                        # a single wide activation evicts both
                        o_ps = pso.tile([128, 2, CN], F32)
                        for h in range(2):
                            t = 2 * p + h
                            col = j * CN + t * 128
                            nc.tensor.matmul(
                                o_ps[:, h, :],
                                kpc[col // 2048][:, col % 2048:col % 2048 + 128],
                                uT[:],
                                start=True, stop=True,
                            )
                        # pure leaky (BN folded into the matmul), PSUM->SBUF
                        nc.scalar.activation(
                            ahat[:, 2 * p:2 * p + 2, :], o_ps[:], AF.Lrelu,
                            scale=1.0, alpha=0.01,
                        )
                    return ahat

                USC = 1.0 / (SA * SW) if Y1_FP8 else 1.0

                def emit_y1(j, ahat):
                    y1 = conv.tile([128, 4, CN], BF16, tag="y1", name=f"y1_{j}")
                    for c1m in range(4):
                        y1_ps = psy1.tile([128, CN], F32)
                        if Y1_FP8:
                            # DoubleRow: one MM consumes a pair of K=128
                            # chunks ([128, 2, M] weights, [128, 2, N] moving)
                            for p in range(2):
                                nc.tensor.matmul(
                                    y1_ps[:],
                                    w1Tc[p][:, :, c1m * 128:(c1m + 1) * 128],
                                    ahat[:, 2 * p:2 * p + 2, :],
                                    start=(p == 0), stop=(p == 1),
                                    perf_mode=mybir.MatmulPerfMode.DoubleRow,
                                )
                        else:
                            for t in range(4):
                                nc.tensor.matmul(
                                    y1_ps[:],
                                    w1Tc[t // 2][:, t % 2, c1m * 128:(c1m + 1) * 128],
                                    ahat[:, t, :],
                                    start=(t == 0), stop=(t == 3),
                                )
                        # last j: scalar takes the LAST two chunks so the
                        # final y2 matmuls aren't gated on the slower 2-op
                        # vector eviction path.
                        on_scalar = (c1m < 2) if j < 7 else (c1m >= 2)
                        if on_scalar:
                            nc.scalar.activation(
                                y1[:, c1m, :], y1_ps[:], AF.Lrelu,
                                bias=b1c[:, c1m:c1m + 1], scale=USC, alpha=0.01,
                            )
                        else:
                            z = sm.tile([128, CN], F32, tag="z")
                            nc.vector.tensor_scalar(
                                z[:], y1_ps[:], USC, b1c[:, c1m:c1m + 1],
                                op0=OP.mult, op1=OP.add,
                            )
                            nc.vector.scalar_tensor_tensor(
                                y1[:, c1m, :], z[:], 0.01, z[:],
                                op0=OP.mult, op1=OP.max,
                            )
                    return y1

                def emit_y2_pair(jA, y1A, jB, y1B):
                    # column-tiled: jA's outputs land in PSUM partitions 0-63
                    # (col group 0), jB's in 64-127 (col group 1); the two
                    # matmul streams run concurrently in the PE array.
                    y2_ps = psy2.tile([128, CN], F32, tag="y2ps",
                                      name=f"y2ps_{jA}")
                    for c1m in range(4):
                        nc.tensor.matmul(
                            y2_ps[0:C, :], w2T[:, c1m, :], y1A[:, c1m, :],
                            start=(c1m == 0), stop=(c1m == 3),
                            tile_position=(0, 0),
                        )
                        nc.tensor.matmul(
                            y2_ps[C:2 * C, :], w2T[:, c1m, :], y1B[:, c1m, :],
                            start=(c1m == 0), stop=(c1m == 3),
                            tile_position=(0, 64),
                        )
                    y2 = conv.tile([128, CN], F32, tag="y2", name=f"y2_{jA}")
                    nc.vector.tensor_scalar(
                        y2[:], y2_ps[:], b2cc[:, 0:1], None, op0=OP.add,
                    )
                    # split the pair's two output DMAs across both DGE queues
                    # so the final pair's stores drain in parallel
                    nc.sync.dma_start(out_d[:, jA * CN:(jA + 1) * CN],
                                      y2[0:C, :])
                    nc.scalar.dma_start(out_d[:, jB * CN:(jB + 1) * CN],
                                        y2[C:2 * C, :])

                ahat = emit_x2(0)
                y1s = {}
                for j in range(8):
                    next_ahat = emit_x2(j + 1) if j < 7 else None
                    if j == 0:
                        # fill the pipeline-head bubble (y1(0) waits on the
                        # ahat(0) evictions) with full-K dummies so the HAM
                        # stays warm into phase C
                        head_ps = psy2.tile([128, CN], F32, tag="y2ps",
                                            name="head_warm")
                        for _ in range(5):
                            nc.tensor.matmul(
                                head_ps[:], ET[:, 0, 0:128], ET[:, 0, :],
                                start=True, stop=True,
                            )
                    y1s[j] = emit_y1(j, ahat)
                    ahat = next_ahat
                    if j % 2 == 0 and j > 0:
                        emit_y2_pair(j - 2, y1s.pop(j - 2), j - 1, y1s.pop(j - 1))
                emit_y2_pair(6, y1s.pop(6), 7, y1s.pop(7))

    nc.finalize()
    return nc


def _get_nc():
    global _nc_cache
    if _nc_cache is None:
        _nc_cache = _build()
    return _nc_cache


def _prepare_in_maps(q, kv, wq, bq, wk, bk, wv, bv,
                     bn_gamma, bn_beta, bn_mean, bn_var, w1, b1, w2, b2):
    f32 = np.float32
    q = np.asarray(q, f32).reshape(B, C, N)
    kv = np.asarray(kv, f32).reshape(B, C, N)
    ones = np.ones((B, 1, N), f32)
    qa = np.concatenate([q, ones], 1)    # [B, 65, N]
    kva = np.concatenate([kv, ones], 1)

    # qa^T / kva^T chunked over n, interleaved: [B, 128, 32, 2, 65]
    qT = qa.transpose(0, 2, 1).reshape(B, NCHUNK, 128, CA)
    kT = kva.transpose(0, 2, 1).reshape(B, NCHUNK, 128, CA)
    qk = np.stack([qT, kT], axis=3).transpose(0, 2, 1, 3, 4)

    bn_scale = (np.asarray(bn_gamma, f32)
                / np.sqrt(np.asarray(bn_var, f32) + np.float32(BN_EPS)))
    bn_shift = np.asarray(bn_beta, f32) - np.asarray(bn_mean, f32) * bn_scale

    # kva with columns permuted: col j*512 + u  <-  original n = 8*u + j;
    # BN scale folded into columns, BN shift as a leading row (pairs with
    # the all-ones row 0 of the normalized U^T on device — row 0 so the
    # softmax-sum reciprocal reads a 32-aligned partition).
    kp = kva.reshape(B, CA, CN, 8).transpose(0, 1, 3, 2).reshape(B, CA, N)
    kp = kp * np.tile(bn_scale, 8)[None, None, :]
    kp = np.concatenate(
        [np.broadcast_to(np.tile(bn_shift, 8), (B, 1, N)), kp], axis=1
    )  # [B, 66, 4096]
    if Y1_FP8:
        kp = kp * np.float32(SA)

    wkaT = np.concatenate([np.asarray(wk, f32), np.asarray(bk, f32)[:, None]], 1).T
    wqaT = (np.concatenate([np.asarray(wq, f32), np.asarray(bq, f32)[:, None]], 1).T
            / np.float32(64.0))                                    # [65, 512]
    wva = np.concatenate(
        [np.ones((CN, 1), f32), np.asarray(wv, f32),
         np.asarray(bv, f32)[:, None]], 1
    ).reshape(4, 128, CB).transpose(1, 0, 2)                       # [128, 4, 66]
    w1T = np.asarray(w1, f32).T.reshape(4, 128, CN).transpose(1, 0, 2)
    if Y1_FP8:
        w1T = np.ascontiguousarray(w1T * np.float32(SW)).astype(
            ml_dtypes.float8_e4m3fn
        )
    else:
        w1T = np.ascontiguousarray(w1T).astype(NPBF16)
    w2T = np.asarray(w2, f32).T.reshape(4, 128, C).transpose(1, 0, 2)

    b1c = np.ascontiguousarray(np.asarray(b1, f32).reshape(4, 128).T)
    b2c = np.tile(np.asarray(b2, f32), 2)[:, None].copy()

    shared = {
        "wkaT": wkaT.astype(NPBF16), "wqaT": wqaT.astype(NPBF16),
        "wva": np.ascontiguousarray(wva).astype(NPBF16),
        "w1T": w1T,
        "w2T": np.ascontiguousarray(w2T).astype(NPBF16),
        "b1c": b1c, "b2c": b2c,
    }
    in_maps = []
    for b in range(B):
        m = dict(shared)
        m["qk"] = np.ascontiguousarray(qk[b]).astype(NPBF16)
        m["kp"] = np.ascontiguousarray(kp[b]).astype(NPBF16)
        in_maps.append(m)
    return in_maps


def _run(in_maps, trace=False):
    nc = _get_nc()
    return run_bass_kernel_spmd(nc, in_maps, list(range(N_CORES)), trace=trace)


def _fetch(res):
    outs = [np.asarray(res.results[i]["out"], np.float32) for i in range(N_CORES)]
    return np.ascontiguousarray(np.stack(outs)).reshape(B, C, H, W)


def kernel(**inputs) -> np.ndarray:
    in_maps = _prepare_in_maps(**inputs)
    # Run twice and compare: guards against rare transient device-state
    # corruption (execution is bitwise deterministic, so a mismatch means
    # one run was corrupted; a third run breaks the tie).
    out1 = _fetch(_run(in_maps, trace=False))
    out2 = _fetch(_run(in_maps, trace=False))
    if np.array_equal(out1, out2):
        return out1
    out3 = _fetch(_run(in_maps, trace=False))
    if np.array_equal(out1, out3):
        return out1
    return out3 if np.array_equal(out2, out3) else out3


def _ensure_ntff_hook():
    """Register antenv.axon_hooks shim so trace=True can NTFF-profile."""
    import sys
    import types
    try:
        import antenv.axon_hooks  # noqa: F401
        return
    except ImportError:
        pass
    from trn_agent_boot.trn_boot import _ntff_profile_via_ctypes
    hook = _ntff_profile_via_ctypes("/opt/axon/libaxon_pjrt.so")
    mod = types.ModuleType("antenv.axon_hooks")
    mod._hook = hook
    mod.get_axon_ntff_profile_hook = lambda: mod._hook
    def _set(h):
        mod._hook = h
    mod.set_axon_ntff_profile_hook = _set
    sys.modules["antenv.axon_hooks"] = mod


def bench(**inputs):
    """Run with NTFF tracing; returns (output, BassKernelResults)."""
    _ensure_ntff_hook()
    in_maps = _prepare_in_maps(**inputs)
    res = _run(in_maps, trace=True)
    outs = [np.asarray(res.results[i]["out"], np.float32) for i in range(N_CORES)]
    return np.stack(outs).reshape(B, C, H, W), res
